# revision 12
# baseline (speedup 1.0000x reference)
import sys

sys.path.insert(0, "/opt/trn_rl_repo")

import numpy as np

import concourse.bacc as bacc
import concourse.bass as bass
import concourse.mybir as mybir
import concourse.tile as tile
from concourse.bass_utils import run_bass_kernel_spmd

# Problem shapes (hardcoded per contract)
B = 4
NQ = 2048
NR = 16384
D = 64
K = 16

NCORES = 8
QPC = NQ // 2          # queries per core (each batch split across 2 cores)
NCHUNK = QPC // 128    # query chunks of 128 per core
MMN = 512              # matmul free dim (one PSUM bank of fp32)
QUAD = 2048            # refs per drain quad (4 PSUM banks)
NQUAD = NR // QUAD     # 8 quads
NWIN = NR // 2         # 8192 width-2 pooled windows per query row
TOPW = 32              # windows kept per query on host (slack over K=16)

_prog_cache = {}


def _build_program(reps: int = 1):
    if reps in _prog_cache:
        return _prog_cache[reps]

    f32 = mybir.dt.float32
    f32r = mybir.dt.float32r
    bf16 = mybir.dt.bfloat16
    mx = mybir.AluOpType.max

    nc = bacc.Bacc("TRN2", target_bir_lowering=False, debug=False, num_devices=NCORES)

    # lhsT rows 0..63 = 2*q^T, row 64 = 1.0, row 65 = q2; rhs rows 0..63 = r^T,
    # row 64 = -r2, row 65 = -1  ->  psum = 2qr - r2 - q2 = -d2
    lhs_d = nc.dram_tensor("lhs", [66, QPC], f32r, kind="ExternalInput")
    rhs_d = nc.dram_tensor("rhs", [66, NR], f32r, kind="ExternalInput")

    # width-2 max-pooled -d2 values; window w = t*1024 + j covers refs
    # t*2048 + j + {0, 1024}
    outP_d = nc.dram_tensor("outP", [QPC, NWIN], bf16, kind="ExternalOutput")

    # small leading pieces so the first quad's matmuls start early
    RHS_CUTS = [0, 1024, 2048, 4096, 8192, 12288, 16384]

    with tile.TileContext(nc) as tc:
        with (
            tc.tile_pool(name="consts", bufs=1) as cpool,
            tc.tile_pool(name="psum", bufs=2, space="PSUM") as ppool,
            tc.tile_pool(name="stage", bufs=3) as spool,
            tc.tile_pool(name="outs", bufs=2) as opool,
        ):
            lhs_t = cpool.tile([66, QPC], f32r)
            nc.sync.dma_start(lhs_t[:, 0:128], lhs_d.ap()[:, 0:128])
            rhs_t = cpool.tile([66, NR], f32r)
            nc.sync.dma_start(
                rhs_t[:, 0:RHS_CUTS[1]], rhs_d.ap()[:, 0:RHS_CUTS[1]]
            )
            nc.sync.dma_start(lhs_t[:, 128:QPC], lhs_d.ap()[:, 128:QPC])
            for p in range(1, len(RHS_CUTS) - 1):
                a, b = RHS_CUTS[p], RHS_CUTS[p + 1]
                nc.sync.dma_start(rhs_t[:, a:b], rhs_d.ap()[:, a:b])

            HQ = QUAD // 2  # 1024 cols per psum operand (2 banks)

            # PE p-state warmup: dummy matmuls on the already-loaded lhs
            # slice while the first rhs piece is still in flight
            ps_w = ppool.tile([128, HQ], f32, tag="pa")
            for w in range(10):
                nc.tensor.matmul(
                    ps_w[:, 0:128], lhs_t[:, 0:128], lhs_t[:, 0:128],
                    start=True, stop=True,
                )

            for rep in range(reps):
              for c in range(NCHUNK):
                lhs_c = lhs_t[:, c * 128:(c + 1) * 128]
                out = opool.tile([128, NWIN], bf16, tag="out")
                for t in range(NQUAD):
                    t0 = t * QUAD
                    psA = ppool.tile([128, HQ], f32, tag="pa")
                    psB = ppool.tile([128, HQ], f32, tag="pb")
                    for h in range(2):
                        nc.tensor.matmul(
                            psA[:, h * MMN:(h + 1) * MMN],
                            lhs_c,
                            rhs_t[:, t0 + h * MMN:t0 + (h + 1) * MMN],
                            start=True,
                            stop=True,
                        )
                    for h in range(2):
                        nc.tensor.matmul(
                            psB[:, h * MMN:(h + 1) * MMN],
                            lhs_c,
                            rhs_t[:, t0 + HQ + h * MMN:t0 + HQ + (h + 1) * MMN],
                            start=True,
                            stop=True,
                        )
                    sA = spool.tile([128, HQ], f32, tag="sa")
                    nc.scalar.copy(sA[:], psA[:])
                    final = (rep == reps - 1) and (c == NCHUNK - 1) and (t == NQUAD - 1)
                    if not final:
                        nc.vector.tensor_tensor(
                            out[:, t * HQ:(t + 1) * HQ], psB[:], sA[:], mx
                        )
                    else:
                        # split the very last drain so its first half can be
                        # shipped out while the second half still runs
                        h2 = HQ // 2
                        nc.vector.tensor_tensor(
                            out[:, t * HQ:t * HQ + h2],
                            psB[:, 0:h2], sA[:, 0:h2], mx,
                        )
                        nc.vector.tensor_tensor(
                            out[:, t * HQ + h2:(t + 1) * HQ],
                            psB[:, h2:HQ], sA[:, h2:HQ], mx,
                        )
                    # stream results out: halves normally; per-quad on the
                    # last chunk so the final transfer is small
                    r0 = c * 128
                    last = (rep == reps - 1) and (c == NCHUNK - 1)
                    if t == NQUAD // 2 - 1 and not last:
                        nc.sync.dma_start(
                            outP_d.ap()[r0:r0 + 128, 0:NWIN // 2],
                            out[:, 0:NWIN // 2],
                        )
                    elif last and t >= NQUAD // 2 - 1:
                        if t == NQUAD // 2 - 1:
                            nc.sync.dma_start(
                                outP_d.ap()[r0:r0 + 128, 0:NWIN // 2],
                                out[:, 0:NWIN // 2],
                            )
                        elif t < NQUAD - 1:
                            w0, w1 = t * HQ, (t + 1) * HQ
                            nc.sync.dma_start(
                                outP_d.ap()[r0:r0 + 128, w0:w1], out[:, w0:w1]
                            )
                        else:
                            # final quad in two halves so the very last
                            # transfer is tiny
                            w0 = t * HQ
                            nc.sync.dma_start(
                                outP_d.ap()[r0:r0 + 128, w0:w0 + HQ // 2],
                                out[:, w0:w0 + HQ // 2],
                            )
                            nc.sync.dma_start(
                                outP_d.ap()[r0:r0 + 128, w0 + HQ // 2:w0 + HQ],
                                out[:, w0 + HQ // 2:w0 + HQ],
                            )
                if not last:
                    nc.sync.dma_start(
                        outP_d.ap()[c * 128:(c + 1) * 128, NWIN // 2:NWIN],
                        out[:, NWIN // 2:NWIN],
                    )

    nc.compile()
    _prog_cache[reps] = nc
    return nc


def kernel(ref: np.ndarray, query: np.ndarray):
    ref = np.asarray(ref, dtype=np.float32)
    query = np.asarray(query, dtype=np.float32)

    # host-side operand prep (layout + norms)
    r2 = np.sum(ref * ref, axis=-1)                      # [B, NR]
    q2 = np.sum(query * query, axis=-1)                  # [B, NQ]
    refT = np.ascontiguousarray(ref.transpose(0, 2, 1))  # [B, D, NR]
    qT = np.ascontiguousarray(query.transpose(0, 2, 1))  # [B, D, NQ]

    nc = _build_program()

    in_maps = []
    for core in range(NCORES):
        b, h = core // 2, core % 2
        lhs = np.empty((66, QPC), dtype=np.float32)
        lhs[0:D, :] = 2.0 * qT[b][:, h * QPC:(h + 1) * QPC]
        lhs[D, :] = 1.0
        lhs[D + 1, :] = q2[b, h * QPC:(h + 1) * QPC]
        rhs = np.empty((66, NR), dtype=np.float32)
        rhs[0:D, :] = refT[b]
        rhs[D, :] = -r2[b]
        rhs[D + 1, :] = -1.0
        in_maps.append({"lhs": lhs, "rhs": rhs})

    res = run_bass_kernel_spmd(nc, in_maps, core_ids=list(range(NCORES)))

    # host-side top-k: pick the best TOPW pooled windows per query (pooled
    # values are bf16 maxima of -d2 over ref pairs), expand to 2*TOPW
    # candidate refs, rescore exactly, take the smallest K.
    Dout = np.empty((B, NQ, K), dtype=np.float32)
    Iout = np.empty((B, NQ, K), dtype=np.int64)
    off = np.array([0, 1024], dtype=np.int64)
    for b in range(B):
        pooled = np.concatenate(
            [
                np.asarray(res.results[2 * b]["outP"]).astype(np.float32),
                np.asarray(res.results[2 * b + 1]["outP"]).astype(np.float32),
            ],
            axis=0,
        )                                                    # [NQ, NWIN]
        widx = np.argpartition(-pooled, TOPW, axis=1)[:, :TOPW]  # [NQ, TOPW]
        base = (widx >> 10) * QUAD + (widx & 1023)
        cand = (base[:, :, None] + off[None, None, :]).reshape(NQ, TOPW * 2)
        cand.sort(axis=1)                                    # id-order for tie-break
        rg = ref[b][cand]                                    # [NQ, TOPW*2, D]
        d2 = (
            q2[b][:, None]
            + r2[b][cand]
            - 2.0 * np.einsum("qd,qkd->qk", query[b], rg, dtype=np.float64)
        )
        ordk = np.argsort(d2, axis=1, kind="stable")[:, :K]
        rows = np.arange(NQ)[:, None]
        d2k = np.maximum(d2[rows, ordk], 0.0)
        Dout[b] = np.sqrt(d2k).astype(np.float32)
        Iout[b] = cand[rows, ordk]
    return (Dout, Iout)


# revision 14
# speedup vs baseline: 1.0062x; 1.0062x over previous
import sys

sys.path.insert(0, "/opt/trn_rl_repo")

import numpy as np

import concourse.bacc as bacc
import concourse.bass as bass
import concourse.mybir as mybir
import concourse.tile as tile
from concourse.bass_utils import run_bass_kernel_spmd

# Problem shapes (hardcoded per contract)
B = 4
NQ = 2048
NR = 16384
D = 64
K = 16

NCORES = 8
QPC = NQ // 2          # queries per core (each batch split across 2 cores)
NCHUNK = QPC // 128    # query chunks of 128 per core
MMN = 512              # matmul free dim (one PSUM bank of fp32)
QUAD = 2048            # refs per drain quad (4 PSUM banks)
NQUAD = NR // QUAD     # 8 quads
NWIN = NR // 2         # 8192 width-2 pooled windows per query row
TOPW = 32              # windows kept per query on host (slack over K=16)

_prog_cache = {}


def _build_program(reps: int = 1):
    if reps in _prog_cache:
        return _prog_cache[reps]

    f32 = mybir.dt.float32
    f32r = mybir.dt.float32r
    bf16 = mybir.dt.bfloat16
    mx = mybir.AluOpType.max

    nc = bacc.Bacc("TRN2", target_bir_lowering=False, debug=False, num_devices=NCORES)

    # lhsT rows 0..63 = 2*q^T, row 64 = 1.0, row 65 = q2; rhs rows 0..63 = r^T,
    # row 64 = -r2, row 65 = -1  ->  psum = 2qr - r2 - q2 = -d2
    lhs_d = nc.dram_tensor("lhs", [66, QPC], f32r, kind="ExternalInput")
    rhs_d = nc.dram_tensor("rhs", [66, NR], f32r, kind="ExternalInput")

    # width-2 max-pooled -d2 values; window w = t*1024 + j covers refs
    # t*2048 + j + {0, 1024}
    outP_d = nc.dram_tensor("outP", [QPC, NWIN], bf16, kind="ExternalOutput")

    # small leading pieces so the first quad's matmuls start early
    RHS_CUTS = [0, 1024, 2048, 4096, 8192, 12288, 16384]

    with tile.TileContext(nc) as tc:
        with (
            tc.tile_pool(name="consts", bufs=1) as cpool,
            tc.tile_pool(name="psum", bufs=2, space="PSUM") as ppool,
            tc.tile_pool(name="stage", bufs=3) as spool,
            tc.tile_pool(name="outs", bufs=2) as opool,
        ):
            lhs_t = cpool.tile([66, QPC], f32r)
            nc.sync.dma_start(lhs_t[:, 0:128], lhs_d.ap()[:, 0:128])
            rhs_t = cpool.tile([66, NR], f32r)
            nc.sync.dma_start(
                rhs_t[:, 0:RHS_CUTS[1]], rhs_d.ap()[:, 0:RHS_CUTS[1]]
            )
            nc.sync.dma_start(lhs_t[:, 128:QPC], lhs_d.ap()[:, 128:QPC])
            for p in range(1, len(RHS_CUTS) - 1):
                a, b = RHS_CUTS[p], RHS_CUTS[p + 1]
                nc.sync.dma_start(rhs_t[:, a:b], rhs_d.ap()[:, a:b])

            HQ = QUAD // 2  # 1024 cols per psum operand (2 banks)

            # PE p-state warmup: dummy matmuls on the already-loaded lhs
            # slice while the first rhs piece is still in flight
            ps_w = ppool.tile([128, HQ], f32, tag="pa")
            for w in range(2):
                nc.tensor.matmul(
                    ps_w[:, 0:128], lhs_t[:, 0:128], lhs_t[:, 0:128],
                    start=True, stop=True,
                )

            for rep in range(reps):
              for c in range(NCHUNK):
                lhs_c = lhs_t[:, c * 128:(c + 1) * 128]
                out = opool.tile([128, NWIN], bf16, tag="out")
                for t in range(NQUAD):
                    t0 = t * QUAD
                    psA = ppool.tile([128, HQ], f32, tag="pa")
                    psB = ppool.tile([128, HQ], f32, tag="pb")
                    for h in range(2):
                        nc.tensor.matmul(
                            psA[:, h * MMN:(h + 1) * MMN],
                            lhs_c,
                            rhs_t[:, t0 + h * MMN:t0 + (h + 1) * MMN],
                            start=True,
                            stop=True,
                        )
                    for h in range(2):
                        nc.tensor.matmul(
                            psB[:, h * MMN:(h + 1) * MMN],
                            lhs_c,
                            rhs_t[:, t0 + HQ + h * MMN:t0 + HQ + (h + 1) * MMN],
                            start=True,
                            stop=True,
                        )
                    sA = spool.tile([128, HQ], f32, tag="sa")
                    nc.scalar.copy(sA[:], psA[:])
                    final = (rep == reps - 1) and (c == NCHUNK - 1) and (t == NQUAD - 1)
                    if not final:
                        nc.vector.tensor_tensor(
                            out[:, t * HQ:(t + 1) * HQ], psB[:], sA[:], mx
                        )
                    else:
                        # split the very last drain so its first half can be
                        # shipped out while the second half still runs
                        h2 = HQ // 2
                        nc.vector.tensor_tensor(
                            out[:, t * HQ:t * HQ + h2],
                            psB[:, 0:h2], sA[:, 0:h2], mx,
                        )
                        nc.vector.tensor_tensor(
                            out[:, t * HQ + h2:(t + 1) * HQ],
                            psB[:, h2:HQ], sA[:, h2:HQ], mx,
                        )
                    # stream results out: halves normally; per-quad on the
                    # last chunk so nothing big queues in front of the tail
                    r0 = c * 128
                    last = (rep == reps - 1) and (c == NCHUNK - 1)
                    if last:
                        if t < NQUAD - 1:
                            w0, w1 = t * HQ, (t + 1) * HQ
                            nc.sync.dma_start(
                                outP_d.ap()[r0:r0 + 128, w0:w1], out[:, w0:w1]
                            )
                        else:
                            w0 = t * HQ
                            nc.sync.dma_start(
                                outP_d.ap()[r0:r0 + 128, w0:w0 + HQ // 2],
                                out[:, w0:w0 + HQ // 2],
                            )
                            nc.sync.dma_start(
                                outP_d.ap()[r0:r0 + 128, w0 + HQ // 2:w0 + HQ],
                                out[:, w0 + HQ // 2:w0 + HQ],
                            )
                    elif t == NQUAD // 2 - 1:
                        nc.sync.dma_start(
                            outP_d.ap()[r0:r0 + 128, 0:NWIN // 2],
                            out[:, 0:NWIN // 2],
                        )
                if not last:
                    nc.sync.dma_start(
                        outP_d.ap()[c * 128:(c + 1) * 128, NWIN // 2:NWIN],
                        out[:, NWIN // 2:NWIN],
                    )

    nc.compile()
    _prog_cache[reps] = nc
    return nc


def kernel(ref: np.ndarray, query: np.ndarray):
    ref = np.asarray(ref, dtype=np.float32)
    query = np.asarray(query, dtype=np.float32)

    # host-side operand prep (layout + norms)
    r2 = np.sum(ref * ref, axis=-1)                      # [B, NR]
    q2 = np.sum(query * query, axis=-1)                  # [B, NQ]
    refT = np.ascontiguousarray(ref.transpose(0, 2, 1))  # [B, D, NR]
    qT = np.ascontiguousarray(query.transpose(0, 2, 1))  # [B, D, NQ]

    nc = _build_program()

    in_maps = []
    for core in range(NCORES):
        b, h = core // 2, core % 2
        lhs = np.empty((66, QPC), dtype=np.float32)
        lhs[0:D, :] = 2.0 * qT[b][:, h * QPC:(h + 1) * QPC]
        lhs[D, :] = 1.0
        lhs[D + 1, :] = q2[b, h * QPC:(h + 1) * QPC]
        rhs = np.empty((66, NR), dtype=np.float32)
        rhs[0:D, :] = refT[b]
        rhs[D, :] = -r2[b]
        rhs[D + 1, :] = -1.0
        in_maps.append({"lhs": lhs, "rhs": rhs})

    res = run_bass_kernel_spmd(nc, in_maps, core_ids=list(range(NCORES)))

    # host-side top-k: pick the best TOPW pooled windows per query (pooled
    # values are bf16 maxima of -d2 over ref pairs), expand to 2*TOPW
    # candidate refs, rescore exactly, take the smallest K.
    Dout = np.empty((B, NQ, K), dtype=np.float32)
    Iout = np.empty((B, NQ, K), dtype=np.int64)
    off = np.array([0, 1024], dtype=np.int64)
    for b in range(B):
        pooled = np.concatenate(
            [
                np.asarray(res.results[2 * b]["outP"]).astype(np.float32),
                np.asarray(res.results[2 * b + 1]["outP"]).astype(np.float32),
            ],
            axis=0,
        )                                                    # [NQ, NWIN]
        widx = np.argpartition(-pooled, TOPW, axis=1)[:, :TOPW]  # [NQ, TOPW]
        base = (widx >> 10) * QUAD + (widx & 1023)
        cand = (base[:, :, None] + off[None, None, :]).reshape(NQ, TOPW * 2)
        cand.sort(axis=1)                                    # id-order for tie-break
        rg = ref[b][cand]                                    # [NQ, TOPW*2, D]
        d2 = (
            q2[b][:, None]
            + r2[b][cand]
            - 2.0 * np.einsum("qd,qkd->qk", query[b], rg, dtype=np.float64)
        )
        ordk = np.argsort(d2, axis=1, kind="stable")[:, :K]
        rows = np.arange(NQ)[:, None]
        d2k = np.maximum(d2[rows, ordk], 0.0)
        Dout[b] = np.sqrt(d2k).astype(np.float32)
        Iout[b] = cand[rows, ordk]
    return (Dout, Iout)
